# revision 10
# baseline (speedup 1.0000x reference)
"""Trainium2 Bass kernel for nn_Blur (upfirdn2d: up=2, pad=(2,1,2,1), 4-tap
separable filter [1,3,3,1] x [1,3,3,1] / 64).

Input  x [16, 128, 128, 128] f32  ->  Output [16, 128, 256, 256] f32.

Math (polyphase decomposition of the zero-insertion upsample + conv):
  per axis, even outputs:  y[2i]   = (1*x[i-1] + 3*x[i]) / 8
            odd  outputs:  y[2i+1] = (3*x[i]   + 1*x[i+1]) / 8
Separable 2D:
  pass 1 (vertical, on TensorE): V = A.T @ X with A the banded [128, 256]
     polyphase matrix carrying the full 1/64 scale. Taps (1/64, 3/64) are
     exact in bf16, so A is stored bf16 (single-pass matmul) while X stays
     fp32 -> result is exact fp32.
     Column order: A[:, i] -> output row 2i (even), A[:, 128+i] -> row 2i+1,
     so PSUM partition i holds output rows 2i and 2i+1 -> 2KB-contiguous
     DRAM chunks on the way out.
  pass 2 (horizontal): out[o,2j] = V[o,j-1] + 3V[o,j],
                       out[o,2j+1] = 3V[o,j] + V[o,j+1]
     with u = 3V on ScalarE and the two adds on VectorE (strided writes).

Sharding: pure data parallel, 2 examples per core x 8 cores. Each core
processes 256 channel-images of [128,128] in groups of 4 (matmul free dim
512).
"""

import numpy as np

H = 128
W = 128
N_CORES = 8
EX_PER_CORE = 2
NIMG_PER_CORE = EX_PER_CORE * 128  # 256 channel-images
GROUP = 4

# Matmul operand mode:
#   "f32"       : lhsT f32, rhs f32 (exact; 2 half-speed MMs per matmul)
#   "f32r"      : operands viewed as float32r (single full-speed matmul;
#                 rounds x to ~11 mantissa bits -> rel err ~1e-4)
#   "f32r_hilo" : x split on-chip into x_hi (f32r-rounded) + x_lo
#                 (remainder); two accumulating f32r matmuls -> ~1e-7
#                 while keeping full-speed PE.
#   "bf16_hilo" : x split into bf16 hi + bf16 lo; 1 cyc/col matmuls and
#                 2-byte weight loads -> rel err ~2.5e-6, least PE time.
MM_MODE = "bf16_hilo"
FILT_BF16 = MM_MODE == "bf16_hilo"


def _filter_matrix() -> np.ndarray:
    """A[h, m]: m in 0..127 -> even output row 2m; m in 128..255 -> odd row
    2(m-128)+1. Carries the full 1/64 scale of the separable pass."""
    A = np.zeros((H, 2 * H), np.float32)
    for i in range(H):
        # even output row 2i = (1*x[i-1] + 3*x[i])/64
        if i - 1 >= 0:
            A[i - 1, i] = 1.0 / 64
        A[i, i] = 3.0 / 64
        # odd output row 2i+1 = (3*x[i] + 1*x[i+1])/64
        A[i, H + i] = 3.0 / 64
        if i + 1 < H:
            A[i + 1, H + i] = 1.0 / 64
    return A


def filter_input() -> np.ndarray:
    A = _filter_matrix()
    if FILT_BF16:
        import ml_dtypes

        A = A.astype(ml_dtypes.bfloat16)
    return A


def build_kernel_body(tc, x, filt, out, nimg):
    """Emit the kernel IR. x [nimg,128,128], filt [128,256], out [nimg,256,256]."""
    from contextlib import ExitStack

    import concourse.mybir as mybir

    f32 = mybir.dt.float32
    f32r = mybir.dt.float32r
    xdt = f32r if MM_MODE == "f32r" else f32
    hdt = mybir.dt.bfloat16 if MM_MODE == "bf16_hilo" else f32r
    fdt = mybir.dt.bfloat16 if FILT_BF16 else (f32 if MM_MODE == "f32" else f32r)
    nc = tc.nc
    ngroups = nimg // GROUP
    GW = GROUP * W  # 512

    with ExitStack() as ctx:
        const_pool = ctx.enter_context(tc.tile_pool(name="const", bufs=1))
        xin_pool = ctx.enter_context(tc.tile_pool(name="xin", bufs=6))
        if MM_MODE in ("f32r_hilo", "bf16_hilo"):
            xh_pool = ctx.enter_context(tc.tile_pool(name="xh", bufs=3))
            xl_pool = ctx.enter_context(tc.tile_pool(name="xl", bufs=3))
        v_pool = ctx.enter_context(tc.tile_pool(name="v", bufs=3, space="PSUM"))
        u_pool = ctx.enter_context(tc.tile_pool(name="u", bufs=4))
        o_pool = ctx.enter_context(tc.tile_pool(name="o", bufs=4))

        A = const_pool.tile([128, 256], fdt)
        filt_src = filt.bitcast(fdt) if fdt == mybir.dt.float32r else filt
        nc.sync.dma_start(A[:], filt_src)

        for g in range(ngroups):
            i0 = g * GROUP
            xg = xin_pool.tile([128, GW], xdt)
            src = x[i0 : i0 + GROUP].rearrange("i h w -> h i w").bitcast(xdt)
            nc.scalar.dma_start(xg[:].rearrange("p (i w) -> p i w", i=GROUP), src)

            # pass 1 (vertical) on TensorE; partition i of v holds:
            #   cols 0:512   = V[2i,   (img, w)]   (even phase)
            #   cols 512:1024= V[2i+1, (img, w)]   (odd phase)
            v = v_pool.tile([128, 2 * GW], f32)
            if MM_MODE in ("f32r_hilo", "bf16_hilo"):
                xh = xh_pool.tile([128, GW], hdt)
                xl = xl_pool.tile([128, GW], hdt)
                nc.scalar.copy(xh[:], xg[:])  # rounds f32 -> f32r
                nc.vector.tensor_sub(xl[:], xg[:], xh[:])
                nc.tensor.matmul(v[:, 0:GW], A[:, 0:128], xh[:], start=True, stop=False)
                nc.tensor.matmul(v[:, 0:GW], A[:, 0:128], xl[:], start=False, stop=True)
                nc.tensor.matmul(
                    v[:, GW : 2 * GW], A[:, 128:256], xh[:], start=True, stop=False
                )
                nc.tensor.matmul(
                    v[:, GW : 2 * GW], A[:, 128:256], xl[:], start=False, stop=True
                )
            else:
                nc.tensor.matmul(v[:, 0:GW], A[:, 0:128], xg[:], start=True, stop=True)
                nc.tensor.matmul(
                    v[:, GW : 2 * GW], A[:, 128:256], xg[:], start=True, stop=True
                )

            # u = 3*V on ScalarE
            u = u_pool.tile([128, 2 * GW], f32)
            nc.scalar.mul(u[:], v[:], 3.0)

            # out tile: partition i = output rows (2i, 2i+1):
            #   layout [img, eo, c] -> (c2 c) contiguous 2KB per (img)
            o = o_pool.tile([128, 2 * GROUP * 2 * W], f32)
            vV = v[:].rearrange("p (eo i w) -> p i eo w", eo=2, i=GROUP)
            uV = u[:].rearrange("p (eo i w) -> p i eo w", eo=2, i=GROUP)
            o4 = o[:].rearrange("p (i eo c) -> p i eo c", i=GROUP, eo=2)

            # interior even cols 2j (j=1..127): V[j-1] + u[j]
            nc.vector.tensor_add(
                o4[:, :, :, 2:255:2], vV[:, :, :, 0:127], uV[:, :, :, 1:128]
            )
            # interior odd cols 2j+1 (j=0..126): u[j] + V[j+1]
            nc.vector.tensor_add(
                o4[:, :, :, 1:254:2], uV[:, :, :, 0:127], vV[:, :, :, 1:128]
            )
            # seams: col 0 = u[0], col 255 = u[127]
            nc.scalar.copy(o4[:, :, :, 0], uV[:, :, :, 0])
            nc.scalar.copy(o4[:, :, :, 255], uV[:, :, :, 127])

            # one DMA for the whole group: partition i -> DRAM rows 2i, 2i+1
            dst = out[i0 : i0 + GROUP].rearrange("i (p c2) c -> p i (c2 c)", c2=2)
            nc.sync.dma_start(dst, o[:].rearrange("p (i cc) -> p i cc", i=GROUP))


def build_bass(nimg=NIMG_PER_CORE, enable_asserts=False):
    import concourse.bacc as bacc
    import concourse.mybir as mybir
    import concourse.tile as tile

    f32 = mybir.dt.float32
    xdt = mybir.dt.float32r if MM_MODE == "f32r" else f32
    fdt = mybir.dt.bfloat16 if FILT_BF16 else (f32 if MM_MODE == "f32" else mybir.dt.float32r)
    nc = bacc.Bacc(
        "TRN2",
        target_bir_lowering=False,
        debug=False,
        enable_asserts=enable_asserts,
        num_devices=N_CORES,
    )
    x = nc.dram_tensor("x", [nimg, H, W], xdt, kind="ExternalInput").ap()
    filt = nc.dram_tensor("filt", [H, 2 * H], fdt, kind="ExternalInput").ap()
    out = nc.dram_tensor("out", [nimg, 2 * H, 2 * W], f32, kind="ExternalOutput").ap()
    with tile.TileContext(nc) as tc:
        build_kernel_body(tc, x, filt, out, nimg)
    nc.compile()
    return nc


_NC_CACHE = {}


def kernel(x: np.ndarray, _trace=False, _trace_cores=None) -> np.ndarray:
    from concourse.bass_utils import run_bass_kernel_spmd

    assert x.shape == (16, 128, H, W), x.shape
    xf = np.ascontiguousarray(x, dtype=np.float32).reshape(N_CORES, NIMG_PER_CORE, H, W)
    A = filter_input()
    in_maps = [{"x": xf[k], "filt": A} for k in range(N_CORES)]

    key = NIMG_PER_CORE
    if key not in _NC_CACHE:
        _NC_CACHE[key] = build_bass()
    nc = _NC_CACHE[key]

    res = run_bass_kernel_spmd(
        nc,
        in_maps,
        core_ids=list(range(N_CORES)),
        trace=_trace,
        trace_cores=_trace_cores,
    )
    outs = np.stack([r["out"] for r in res.results])  # [8, 256, 256, 256]
    out = outs.reshape(16, 128, 2 * H, 2 * W)
    if _trace:
        kernel._last_result = res
    return out


# revision 11
# speedup vs baseline: 1.1194x; 1.1194x over previous
"""Trainium2 Bass kernel for nn_Blur (upfirdn2d: up=2, pad=(2,1,2,1), 4-tap
separable filter [1,3,3,1] x [1,3,3,1] / 64).

Input  x [16, 128, 128, 128] f32  ->  Output [16, 128, 256, 256] f32.

Math (polyphase decomposition of the zero-insertion upsample + conv):
  per axis, even outputs:  y[2i]   = (1*x[i-1] + 3*x[i]) / 8
            odd  outputs:  y[2i+1] = (3*x[i]   + 1*x[i+1]) / 8
Separable 2D:
  pass 1 (vertical, on TensorE): V = A.T @ X with A the banded [128, 256]
     polyphase matrix carrying the full 1/64 scale. Taps (1/64, 3/64) are
     exact in bf16, so A is stored bf16 (single-pass matmul) while X stays
     fp32 -> result is exact fp32.
     Column order: A[:, i] -> output row 2i (even), A[:, 128+i] -> row 2i+1,
     so PSUM partition i holds output rows 2i and 2i+1 -> 2KB-contiguous
     DRAM chunks on the way out.
  pass 2 (horizontal): out[o,2j] = V[o,j-1] + 3V[o,j],
                       out[o,2j+1] = 3V[o,j] + V[o,j+1]
     with u = 3V on ScalarE and the two adds on VectorE (strided writes).

Sharding: pure data parallel, 2 examples per core x 8 cores. Each core
processes 256 channel-images of [128,128] in groups of 4 (matmul free dim
512).
"""

import numpy as np

H = 128
W = 128
N_CORES = 8
EX_PER_CORE = 2
NIMG_PER_CORE = EX_PER_CORE * 128  # 256 channel-images
GROUP = 4

# Matmul operand mode:
#   "f32"       : lhsT f32, rhs f32 (exact; 2 half-speed MMs per matmul)
#   "f32r"      : operands viewed as float32r (single full-speed matmul;
#                 rounds x to ~11 mantissa bits -> rel err ~1e-4)
#   "f32r_hilo" : x split on-chip into x_hi (f32r-rounded) + x_lo
#                 (remainder); two accumulating f32r matmuls -> ~1e-7
#                 while keeping full-speed PE.
#   "bf16_hilo" : x split into bf16 hi + bf16 lo; 1 cyc/col matmuls and
#                 2-byte weight loads -> rel err ~2.5e-6, least PE time.
MM_MODE = "bf16_hilo"
FILT_BF16 = MM_MODE == "bf16_hilo"


def _filter_matrix() -> np.ndarray:
    """A[h, m]: m in 0..127 -> even output row 2m; m in 128..255 -> odd row
    2(m-128)+1. Carries the full 1/64 scale of the separable pass."""
    A = np.zeros((H, 2 * H), np.float32)
    for i in range(H):
        # even output row 2i = (1*x[i-1] + 3*x[i])/64
        if i - 1 >= 0:
            A[i - 1, i] = 1.0 / 64
        A[i, i] = 3.0 / 64
        # odd output row 2i+1 = (3*x[i] + 1*x[i+1])/64
        A[i, H + i] = 3.0 / 64
        if i + 1 < H:
            A[i + 1, H + i] = 1.0 / 64
    return A


def filter_input() -> np.ndarray:
    A = _filter_matrix()
    if FILT_BF16:
        import ml_dtypes

        A = A.astype(ml_dtypes.bfloat16)
    return A


def build_kernel_body(tc, x, filt, out, nimg):
    """Emit the kernel IR. x [nimg,128,128], filt [128,256], out [nimg,256,256]."""
    from contextlib import ExitStack

    import concourse.mybir as mybir

    f32 = mybir.dt.float32
    f32r = mybir.dt.float32r
    xdt = f32r if MM_MODE == "f32r" else f32
    hdt = mybir.dt.bfloat16 if MM_MODE == "bf16_hilo" else f32r
    fdt = mybir.dt.bfloat16 if FILT_BF16 else (f32 if MM_MODE == "f32" else f32r)
    nc = tc.nc
    ngroups = nimg // GROUP
    GW = GROUP * W  # 512

    with ExitStack() as ctx:
        const_pool = ctx.enter_context(tc.tile_pool(name="const", bufs=1))
        xin_pool = ctx.enter_context(tc.tile_pool(name="xin", bufs=8))
        if MM_MODE in ("f32r_hilo", "bf16_hilo"):
            xh_pool = ctx.enter_context(tc.tile_pool(name="xh", bufs=3))
            xl_pool = ctx.enter_context(tc.tile_pool(name="xl", bufs=3))
        v_pool = ctx.enter_context(tc.tile_pool(name="v", bufs=4, space="PSUM"))
        u_pool = ctx.enter_context(tc.tile_pool(name="u", bufs=4))
        o_pool = ctx.enter_context(tc.tile_pool(name="o", bufs=4))

        A = const_pool.tile([128, 256], fdt)
        filt_src = filt.bitcast(fdt) if fdt == mybir.dt.float32r else filt
        nc.sync.dma_start(A[:], filt_src)

        for g in range(ngroups):
            i0 = g * GROUP
            xg = xin_pool.tile([128, GW], xdt)
            src = x[i0 : i0 + GROUP].rearrange("i h w -> h i w").bitcast(xdt)
            nc.gpsimd.dma_start(xg[:].rearrange("p (i w) -> p i w", i=GROUP), src)

            # pass 1 (vertical) on TensorE; partition i of v holds:
            #   cols 0:512   = V[2i,   (img, w)]   (even phase)
            #   cols 512:1024= V[2i+1, (img, w)]   (odd phase)
            v = v_pool.tile([128, 2 * GW], f32)
            if MM_MODE in ("f32r_hilo", "bf16_hilo"):
                xh = xh_pool.tile([128, GW], hdt)
                xl = xl_pool.tile([128, GW], hdt)
                nc.scalar.copy(xh[:], xg[:])  # rounds f32 -> f32r
                nc.vector.tensor_sub(xl[:], xg[:], xh[:])
                nc.tensor.matmul(v[:, 0:GW], A[:, 0:128], xh[:], start=True, stop=False)
                nc.tensor.matmul(v[:, 0:GW], A[:, 0:128], xl[:], start=False, stop=True)
                nc.tensor.matmul(
                    v[:, GW : 2 * GW], A[:, 128:256], xh[:], start=True, stop=False
                )
                nc.tensor.matmul(
                    v[:, GW : 2 * GW], A[:, 128:256], xl[:], start=False, stop=True
                )
            else:
                nc.tensor.matmul(v[:, 0:GW], A[:, 0:128], xg[:], start=True, stop=True)
                nc.tensor.matmul(
                    v[:, GW : 2 * GW], A[:, 128:256], xg[:], start=True, stop=True
                )

            # u = 3*V on ScalarE
            u = u_pool.tile([128, 2 * GW], f32)
            nc.scalar.mul(u[:], v[:], 3.0)

            # out tile: partition i = output rows (2i, 2i+1):
            #   layout [img, eo, c] -> (c2 c) contiguous 2KB per (img)
            o = o_pool.tile([128, 2 * GROUP * 2 * W], f32)
            vV = v[:].rearrange("p (eo i w) -> p i eo w", eo=2, i=GROUP)
            uV = u[:].rearrange("p (eo i w) -> p i eo w", eo=2, i=GROUP)
            o4 = o[:].rearrange("p (i eo c) -> p i eo c", i=GROUP, eo=2)

            # interior even cols 2j (j=1..127): V[j-1] + u[j]
            nc.vector.tensor_add(
                o4[:, :, :, 2:255:2], vV[:, :, :, 0:127], uV[:, :, :, 1:128]
            )
            # interior odd cols 2j+1 (j=0..126): u[j] + V[j+1]
            nc.vector.tensor_add(
                o4[:, :, :, 1:254:2], uV[:, :, :, 0:127], vV[:, :, :, 1:128]
            )
            # seams: col 0 = u[0], col 255 = u[127]
            nc.scalar.copy(o4[:, :, :, 0], uV[:, :, :, 0])
            nc.scalar.copy(o4[:, :, :, 255], uV[:, :, :, 127])

            # one DMA for the whole group: partition i -> DRAM rows 2i, 2i+1
            # alternate between the two HWDGE rings (SP / ACT issuers)
            dst = out[i0 : i0 + GROUP].rearrange("i (p c2) c -> p i (c2 c)", c2=2)
            out_eng = nc.sync if g % 2 == 0 else nc.scalar
            out_eng.dma_start(dst, o[:].rearrange("p (i cc) -> p i cc", i=GROUP))


def build_bass(nimg=NIMG_PER_CORE, enable_asserts=False):
    import concourse.bacc as bacc
    import concourse.mybir as mybir
    import concourse.tile as tile

    f32 = mybir.dt.float32
    xdt = mybir.dt.float32r if MM_MODE == "f32r" else f32
    fdt = mybir.dt.bfloat16 if FILT_BF16 else (f32 if MM_MODE == "f32" else mybir.dt.float32r)
    nc = bacc.Bacc(
        "TRN2",
        target_bir_lowering=False,
        debug=False,
        enable_asserts=enable_asserts,
        num_devices=N_CORES,
    )
    x = nc.dram_tensor("x", [nimg, H, W], xdt, kind="ExternalInput").ap()
    filt = nc.dram_tensor("filt", [H, 2 * H], fdt, kind="ExternalInput").ap()
    out = nc.dram_tensor("out", [nimg, 2 * H, 2 * W], f32, kind="ExternalOutput").ap()
    with tile.TileContext(nc) as tc:
        build_kernel_body(tc, x, filt, out, nimg)
    nc.compile()
    return nc


_NC_CACHE = {}


def kernel(x: np.ndarray, _trace=False, _trace_cores=None) -> np.ndarray:
    from concourse.bass_utils import run_bass_kernel_spmd

    assert x.shape == (16, 128, H, W), x.shape
    xf = np.ascontiguousarray(x, dtype=np.float32).reshape(N_CORES, NIMG_PER_CORE, H, W)
    A = filter_input()
    in_maps = [{"x": xf[k], "filt": A} for k in range(N_CORES)]

    key = NIMG_PER_CORE
    if key not in _NC_CACHE:
        _NC_CACHE[key] = build_bass()
    nc = _NC_CACHE[key]

    res = run_bass_kernel_spmd(
        nc,
        in_maps,
        core_ids=list(range(N_CORES)),
        trace=_trace,
        trace_cores=_trace_cores,
    )
    outs = np.stack([r["out"] for r in res.results])  # [8, 256, 256, 256]
    out = outs.reshape(16, 128, 2 * H, 2 * W)
    if _trace:
        kernel._last_result = res
    return out
